# revision 2
# baseline (speedup 1.0000x reference)
"""Cross-entropy loss kernel for Trainium2 (8 NeuronCores, Bass/Tile).

loss = mean_r [ logsumexp(logits[r, :]) - logits[r, labels[r]] ]

Sharding: rows (N) split evenly across 8 cores (data parallel). Each core
streams its [32768, 1000] f32 shard HBM->SBUF once (memory-bound part),
computes per-row -max (DVE), exp(x-max) with accumulated per-row sum (ACT,
one pass), gathers the label logit per row with an indirect DMA, and reduces
(logsumexp - picked) to a per-partition partial [128, 1]. The host sums the
8x128 partials in float64 and divides by N.
"""

import sys

import numpy as np

sys.path.insert(0, "/opt/trn_rl_repo")

N, C = 262144, 1000
NCORES = 8
NSH = N // NCORES  # rows per core = 32768
P = 128  # SBUF partitions

_cache = {}


def _build(nsh, kk, bufs):
    """Build + compile the per-core Bass program.

    nsh: rows handled by one core (divisible by 128*kk)
    kk:  rows per partition per stream tile
    """
    key = (nsh, kk, bufs)
    if key in _cache:
        return _cache[key]

    import concourse.bacc as bacc
    import concourse.bass as bass
    import concourse.tile as tile
    from concourse import mybir

    f32 = mybir.dt.float32
    j = nsh // P          # rows per partition
    t_count = j // kk     # number of stream tiles
    tile_f = kk * C       # free-dim elements per stream tile

    nc = bacc.Bacc("TRN2", target_bir_lowering=False, debug=False)
    logits = nc.dram_tensor("logits", [nsh * C], f32, kind="ExternalInput")
    lidx = nc.dram_tensor("lidx", [P, j], mybir.dt.int32, kind="ExternalInput")
    partial = nc.dram_tensor("partial", [P, 1], f32, kind="ExternalOutput")

    # partition p holds rows [p*j, (p+1)*j): contiguous 1 MB per partition
    stream = logits[:].rearrange("(p m) -> p m", p=P)  # [128, j*C]
    table = logits[:].rearrange("(m o) -> m o", o=1)   # [nsh*C, 1]

    with tile.TileContext(nc) as tc:
        with (
            tc.tile_pool(name="big", bufs=bufs) as big,
            tc.tile_pool(name="acc", bufs=1) as acc,
        ):
            idx_t = acc.tile([P, j], mybir.dt.int32)
            nc.sync.dma_start(out=idx_t[:], in_=lidx[:])
            picked = acc.tile([P, j], f32)
            nc.gpsimd.indirect_dma_start(
                out=picked[:],
                out_offset=None,
                in_=table,
                in_offset=bass.IndirectOffsetOnAxis(ap=idx_t[:], axis=0),
            )

            negmax = acc.tile([P, j], f32)
            sums = acc.tile([P, j], f32)
            for t in range(t_count):
                xt = big.tile([P, tile_f], f32)
                nc.sync.dma_start(
                    out=xt[:], in_=stream[:, t * tile_f : (t + 1) * tile_f]
                )
                nc.vector.reduce_max(
                    out=negmax[:, t * kk : (t + 1) * kk],
                    in_=xt[:].rearrange("p (k c) -> p k c", k=kk),
                    axis=mybir.AxisListType.X,
                    op=mybir.AluOpType.max,
                    negate=True,
                )
                for k in range(kk):
                    jj = t * kk + k
                    nc.scalar.activation(
                        out=xt[:, k * C : (k + 1) * C],
                        in_=xt[:, k * C : (k + 1) * C],
                        func=mybir.ActivationFunctionType.Exp,
                        bias=negmax[:, jj : jj + 1],
                        accum_out=sums[:, jj : jj + 1],
                    )

            # logsumexp = ln(sums) + max = ln(sums) - negmax
            lse = acc.tile([P, j], f32)
            nc.scalar.activation(
                out=lse[:], in_=sums[:], func=mybir.ActivationFunctionType.Ln
            )
            nc.vector.tensor_sub(lse[:], lse[:], negmax[:])
            nc.vector.tensor_sub(lse[:], lse[:], picked[:])
            red = acc.tile([P, 1], f32)
            nc.vector.reduce_sum(
                out=red[:], in_=lse[:], axis=mybir.AxisListType.X,
                op=mybir.AluOpType.add,
            )
            nc.sync.dma_start(out=partial[:], in_=red[:])

    nc.compile()
    _cache[key] = nc
    return nc


def _make_in_maps(logits, labels, ncores, nsh):
    logits = np.ascontiguousarray(np.asarray(logits), dtype=np.float32)
    labels = np.asarray(labels).astype(np.int64)
    j = nsh // P
    in_maps = []
    for cix in range(ncores):
        sh = logits[cix * nsh : (cix + 1) * nsh]
        lab = labels[cix * nsh : (cix + 1) * nsh]
        flat = np.arange(nsh, dtype=np.int64) * C + lab  # shard-local flat index
        in_maps.append(
            {
                "logits": sh.reshape(-1),
                "lidx": flat.reshape(P, j).astype(np.int32),
            }
        )
    return in_maps


def _install_ntff_hook():
    """The agent image's antenv lacks axon_hooks; supply it so
    run_bass_kernel_spmd(trace=True) can drive NTFF profiling through
    the ctypes hook that trn_boot ships."""
    import types

    if "antenv.axon_hooks" in sys.modules:
        return
    try:
        from trn_agent_boot.trn_boot import _ntff_profile_via_ctypes
    except ImportError:
        return
    hook = _ntff_profile_via_ctypes("/opt/axon/libaxon_pjrt.so")
    mod = types.ModuleType("antenv.axon_hooks")
    state = {"h": hook}
    mod.set_axon_ntff_profile_hook = lambda h: state.__setitem__("h", h)
    mod.get_axon_ntff_profile_hook = lambda: state["h"]
    sys.modules["antenv.axon_hooks"] = mod


def run(logits, labels, kk=8, bufs=3, trace=False):
    """Returns (loss, exec_time_ns or None)."""
    from concourse.bass_utils import run_bass_kernel_spmd

    if trace:
        _install_ntff_hook()

    nc = _build(NSH, kk, bufs)
    in_maps = _make_in_maps(logits, labels, NCORES, NSH)
    res = run_bass_kernel_spmd(
        nc, in_maps, core_ids=list(range(NCORES)), trace=trace
    )
    tot = 0.0
    for r in res.results:
        tot += float(np.sum(np.asarray(r["partial"]).astype(np.float64)))
    return np.float32(tot / N), res.exec_time_ns


def kernel(logits, labels):
    loss, _ = run(logits, labels)
    return loss
